# revision 1
# baseline (speedup 1.0000x reference)
"""MoE (16 routed experts, top-2, + shared expert) on 8 TRN2 NeuronCores.

Strategy (expert-parallel per the sharding hint):
  Launch A (SPMD, data-parallel over tokens): each core takes a 2048-token
    slice, computes router logits/softmax/top-2 combine weights on-device
    (fp32 matmul for exact-ish selection) and the shared-expert SwiGLU FFN
    (fp32r matmuls = bf16-speed).  Outputs: comb (2048x16), shared y^T.
  Host: reads comb, builds per-expert token index lists, gathers token
    vectors into dense per-expert batches (the "all-to-all dispatch").
  Launch B (SPMD, expert-parallel): core c owns experts 2c and 2c+1; runs
    the SwiGLU FFN on each expert's gathered batch, scaling rows by the
    combine weight on-device.  Outputs: weighted y^T per expert.
  Host: scatter-adds expert outputs + shared outputs into the full result
    (the "combine").

All activations travel transposed (feature-major, token-minor) so every
matmul operand loads with natural DMA strides and zero on-device transposes.
"""

import math

import numpy as np

# model dims (fixed for this problem)
E, TOPK, C, I = 16, 2, 768, 1536
B, T = 8, 2048
NCORE = 8
NTOK = B * T           # 16384
TPC = NTOK // NCORE    # 2048 tokens per core
CK = C // 128          # 6 contraction chunks for C
IK = I // 128          # 12 chunks for I
NBLK = 512             # token block = PE moving-dim per matmul

TRACE = False          # set True (from a driver) to capture NTFF timing
LAST = {}              # timing info from the most recent kernel() call

_progs = {}            # compiled program cache


def _enable_axon_ntff_profiling():
    import sys
    import types

    if "antenv.axon_hooks" not in sys.modules:
        mod = types.ModuleType("antenv.axon_hooks")
        mod._hook = None
        mod.set_axon_ntff_profile_hook = lambda h: setattr(mod, "_hook", h)
        mod.get_axon_ntff_profile_hook = lambda: mod._hook
        sys.modules["antenv.axon_hooks"] = mod
    from antenv.axon_hooks import set_axon_ntff_profile_hook  # type: ignore
    from trn_agent_boot.trn_boot import _ntff_profile_via_ctypes

    set_axon_ntff_profile_hook(_ntff_profile_via_ctypes("/opt/axon/libaxon_pjrt.so"))
    import concourse.bass_utils as bu

    bu.upload_artifacts = lambda tmpdir: f"file://{tmpdir}"


def _blocks(m):
    """Split m tokens into PE-friendly blocks (<=512 each)."""
    out = []
    n0 = 0
    while n0 < m:
        nb = min(NBLK, m - n0)
        out.append((n0, nb))
        n0 += nb
    return out


def _emit_ffn_block(nc, pools, x_all, wg_sb, wu_sb, wd_sb, scale_sb, y_ap, n0, nblk):
    """One token-block of SwiGLU FFN in transposed layout.

    x_all: SBUF [128, CK, NBLK] (c-major, token-minor) for this block
    wg_sb/wu_sb: SBUF [128, CK, I]; wd_sb: SBUF [128, IK, C]
    scale_sb: SBUF [128, cap] per-token combine weight (or None)
    y_ap: DRAM (C, M) output, written at columns [n0, n0+nblk)
    """
    import concourse.mybir as mybir

    f32 = mybir.dt.float32
    f32r = mybir.dt.float32r
    hpool, gpool, ypool, pgu, pd = (
        pools["h"],
        pools["g"],
        pools["y"],
        pools["pgu"],
        pools["pd"],
    )

    h_all = hpool.tile([128, IK, NBLK], f32r, tag="h_all")
    for ik in range(IK):
        psg = pgu.tile([128, NBLK], f32, tag="psg")
        psu = pgu.tile([128, NBLK], f32, tag="psu")
        for ck in range(CK):
            nc.tensor.matmul(
                psg[:, :nblk],
                lhsT=wg_sb[:, ck, ik * 128 : (ik + 1) * 128],
                rhs=x_all[:, ck, :nblk],
                start=(ck == 0),
                stop=(ck == CK - 1),
            )
        for ck in range(CK):
            nc.tensor.matmul(
                psu[:, :nblk],
                lhsT=wu_sb[:, ck, ik * 128 : (ik + 1) * 128],
                rhs=x_all[:, ck, :nblk],
                start=(ck == 0),
                stop=(ck == CK - 1),
            )
        ga = gpool.tile([128, NBLK], f32, tag="ga")
        nc.scalar.activation(
            ga[:, :nblk], psg[:, :nblk], mybir.ActivationFunctionType.Silu
        )
        nc.vector.tensor_mul(h_all[:, ik, :nblk], ga[:, :nblk], psu[:, :nblk])

    for ck in range(CK):
        psd = pd.tile([128, NBLK], f32, tag="psd")
        for ik in range(IK):
            nc.tensor.matmul(
                psd[:, :nblk],
                lhsT=wd_sb[:, ik, ck * 128 : (ck + 1) * 128],
                rhs=h_all[:, ik, :nblk],
                start=(ik == 0),
                stop=(ik == IK - 1),
            )
        yb = ypool.tile([128, NBLK], f32, tag="yb")
        if scale_sb is None:
            nc.vector.tensor_copy(yb[:, :nblk], psd[:, :nblk])
        else:
            nc.vector.tensor_mul(
                yb[:, :nblk], psd[:, :nblk], scale_sb[:, n0 : n0 + nblk]
            )
        nc.sync.dma_start(
            out=y_ap[ck * 128 : (ck + 1) * 128, n0 : n0 + nblk], in_=yb[:, :nblk]
        )


def _build_launch_a():
    """Router + shared expert, one 2048-token slice per core."""
    from contextlib import ExitStack

    import concourse.tile as tile
    from concourse import bacc, mybir

    f32 = mybir.dt.float32
    AX = mybir.AxisListType.X
    OP = mybir.AluOpType

    nc = bacc.Bacc("TRN2", target_bir_lowering=False, debug=False)
    f32r = mybir.dt.float32r
    xt_ap = nc.dram_tensor("xt", [C, TPC], f32, kind="ExternalInput").ap()
    wgate_ap = nc.dram_tensor("wgate", [C, E], f32, kind="ExternalInput").ap()
    biasb_ap = nc.dram_tensor("biasb", [128, E], f32, kind="ExternalInput").ap()
    swg_ap = nc.dram_tensor("swg", [C, I], f32r, kind="ExternalInput").ap()
    swu_ap = nc.dram_tensor("swu", [C, I], f32r, kind="ExternalInput").ap()
    swd_ap = nc.dram_tensor("swd", [I, C], f32r, kind="ExternalInput").ap()
    comb_ap = nc.dram_tensor("comb", [TPC, E], f32, kind="ExternalOutput").ap()
    yst_ap = nc.dram_tensor("yst", [C, TPC], f32, kind="ExternalOutput").ap()

    with tile.TileContext(nc) as tc, ExitStack() as ctx:
        wpool = ctx.enter_context(tc.tile_pool(name="weights", bufs=1))
        xpool = ctx.enter_context(tc.tile_pool(name="xp", bufs=2))
        hpool = ctx.enter_context(tc.tile_pool(name="hp", bufs=1))
        gpool = ctx.enter_context(tc.tile_pool(name="gp", bufs=2))
        ypool = ctx.enter_context(tc.tile_pool(name="yp", bufs=3))
        rpool = ctx.enter_context(tc.tile_pool(name="rp", bufs=2))
        pgu = ctx.enter_context(tc.tile_pool(name="pgu", bufs=2, space="PSUM"))
        pd = ctx.enter_context(tc.tile_pool(name="pd", bufs=2, space="PSUM"))
        pr = ctx.enter_context(tc.tile_pool(name="pr", bufs=2, space="PSUM"))
        xrpool = ctx.enter_context(tc.tile_pool(name="xr", bufs=1))
        pools = {"h": hpool, "g": gpool, "y": ypool, "pgu": pgu, "pd": pd}

        wgate_sb = wpool.tile([128, CK, E], f32, tag="wgate")
        swg_sb = wpool.tile([128, CK, I], f32r, tag="swg")
        swu_sb = wpool.tile([128, CK, I], f32r, tag="swu")
        swd_sb = wpool.tile([128, IK, C], f32r, tag="swd")
        bias_sb = wpool.tile([128, E], f32, tag="bias")
        for ck in range(CK):
            nc.sync.dma_start(
                out=swg_sb[:, ck, :], in_=swg_ap[ck * 128 : (ck + 1) * 128, :]
            )
        for ck in range(CK):
            nc.sync.dma_start(
                out=wgate_sb[:, ck, :], in_=wgate_ap[ck * 128 : (ck + 1) * 128, :]
            )
        nc.sync.dma_start(out=bias_sb[:], in_=biasb_ap[:])
        for ck in range(CK):
            nc.sync.dma_start(
                out=swu_sb[:, ck, :], in_=swu_ap[ck * 128 : (ck + 1) * 128, :]
            )
        for ik in range(IK):
            nc.sync.dma_start(
                out=swd_sb[:, ik, :], in_=swd_ap[ik * 128 : (ik + 1) * 128, :]
            )

        for n in range(TPC // NBLK):
            x32 = xpool.tile([128, CK, NBLK], f32, tag="x32")
            for ck in range(CK):
                nc.sync.dma_start(
                    out=x32[:, ck, :],
                    in_=xt_ap[ck * 128 : (ck + 1) * 128, n * NBLK : (n + 1) * NBLK],
                )
            x_all = xrpool.tile([128, CK, NBLK], f32r, tag="x_all")
            nc.vector.tensor_copy(x_all[:], x32[:])
            # router: tokens as PSUM partitions, 4 chunks of 128 per block
            for q in range(NBLK // 128):
                t0 = q * 128
                psl = pr.tile([128, E], f32, tag="psl")
                for ck in range(CK):
                    nc.tensor.matmul(
                        psl[:],
                        lhsT=x32[:, ck, t0 : t0 + 128],
                        rhs=wgate_sb[:, ck, :],
                        start=(ck == 0),
                        stop=(ck == CK - 1),
                    )
                lg = rpool.tile([128, E], f32, tag="lg")
                nc.vector.tensor_add(lg[:], psl[:], bias_sb[:])
                m1 = rpool.tile([128, 1], f32, tag="m1")
                nc.vector.reduce_max(m1[:], lg[:], axis=AX)
                nm1 = rpool.tile([128, 1], f32, tag="nm1")
                nc.vector.tensor_scalar_mul(nm1[:], m1[:], -1.0)
                ex = rpool.tile([128, E], f32, tag="ex")
                nc.scalar.activation(
                    ex[:], lg[:], mybir.ActivationFunctionType.Exp, bias=nm1[:]
                )
                msk1 = rpool.tile([128, E], f32, tag="msk1")
                nc.vector.tensor_scalar(msk1[:], lg[:], m1[:], None, op0=OP.is_equal)
                pen = rpool.tile([128, E], f32, tag="pen")
                nc.vector.tensor_scalar_mul(pen[:], msk1[:], 1e30)
                lm = rpool.tile([128, E], f32, tag="lm")
                nc.vector.tensor_sub(lm[:], lg[:], pen[:])
                m2 = rpool.tile([128, 1], f32, tag="m2")
                nc.vector.reduce_max(m2[:], lm[:], axis=AX)
                ge = rpool.tile([128, E], f32, tag="ge")
                nc.vector.tensor_scalar(ge[:], lg[:], m2[:], None, op0=OP.is_ge)
                we = rpool.tile([128, E], f32, tag="we")
                nc.vector.tensor_mul(we[:], ex[:], ge[:])
                sm = rpool.tile([128, 1], f32, tag="sm")
                nc.vector.reduce_sum(sm[:], we[:], axis=AX)
                rs = rpool.tile([128, 1], f32, tag="rs")
                nc.vector.reciprocal(rs[:], sm[:])
                cmb = rpool.tile([128, E], f32, tag="cmb")
                nc.vector.tensor_scalar(cmb[:], we[:], rs[:], None, op0=OP.mult)
                nc.sync.dma_start(
                    out=comb_ap[n * NBLK + t0 : n * NBLK + t0 + 128, :], in_=cmb[:]
                )
            # shared expert FFN on this block
            _emit_ffn_block(
                nc, pools, x_all, swg_sb, swu_sb, swd_sb, None, yst_ap, n * NBLK, NBLK
            )

    nc.compile()
    return nc


def _build_launch_b(cap):
    """Two routed experts per core on dense gathered batches of size cap."""
    from contextlib import ExitStack

    import concourse.tile as tile
    from concourse import bacc, mybir

    f32 = mybir.dt.float32
    f32r = mybir.dt.float32r

    nc = bacc.Bacc("TRN2", target_bir_lowering=False, debug=False)
    aps = {}
    for s in ("a", "b"):
        aps[f"x{s}"] = nc.dram_tensor(f"x{s}t", [C, cap], f32r, kind="ExternalInput").ap()
        aps[f"wg{s}"] = nc.dram_tensor(f"wg{s}", [C, I], f32r, kind="ExternalInput").ap()
        aps[f"wu{s}"] = nc.dram_tensor(f"wu{s}", [C, I], f32r, kind="ExternalInput").ap()
        aps[f"wd{s}"] = nc.dram_tensor(f"wd{s}", [I, C], f32r, kind="ExternalInput").ap()
        aps[f"sc{s}"] = nc.dram_tensor(f"sc{s}", [128, cap], f32, kind="ExternalInput").ap()
        aps[f"y{s}"] = nc.dram_tensor(f"y{s}t", [C, cap], f32, kind="ExternalOutput").ap()

    with tile.TileContext(nc) as tc, ExitStack() as ctx:
        wpool = ctx.enter_context(tc.tile_pool(name="weights", bufs=1))
        xpool = ctx.enter_context(tc.tile_pool(name="xp", bufs=2))
        hpool = ctx.enter_context(tc.tile_pool(name="hp", bufs=1))
        gpool = ctx.enter_context(tc.tile_pool(name="gp", bufs=2))
        ypool = ctx.enter_context(tc.tile_pool(name="yp", bufs=3))
        spool = ctx.enter_context(tc.tile_pool(name="sp", bufs=1))
        pgu = ctx.enter_context(tc.tile_pool(name="pgu", bufs=2, space="PSUM"))
        pd = ctx.enter_context(tc.tile_pool(name="pd", bufs=2, space="PSUM"))
        pools = {"h": hpool, "g": gpool, "y": ypool, "pgu": pgu, "pd": pd}

        for s in ("a", "b"):
            wg_sb = wpool.tile([128, CK, I], f32r, tag="wg")
            wu_sb = wpool.tile([128, CK, I], f32r, tag="wu")
            wd_sb = wpool.tile([128, IK, C], f32r, tag="wd")
            sc_sb = spool.tile([128, cap], f32, tag="sc")
            for ck in range(CK):
                nc.sync.dma_start(
                    out=wg_sb[:, ck, :], in_=aps[f"wg{s}"][ck * 128 : (ck + 1) * 128, :]
                )
            for ck in range(CK):
                nc.sync.dma_start(
                    out=wu_sb[:, ck, :], in_=aps[f"wu{s}"][ck * 128 : (ck + 1) * 128, :]
                )
            for ik in range(IK):
                nc.sync.dma_start(
                    out=wd_sb[:, ik, :], in_=aps[f"wd{s}"][ik * 128 : (ik + 1) * 128, :]
                )
            nc.sync.dma_start(out=sc_sb[:], in_=aps[f"sc{s}"][:])
            for n0, nblk in _blocks(cap):
                x_all = xpool.tile([128, CK, NBLK], f32r, tag="x_all")
                for ck in range(CK):
                    nc.sync.dma_start(
                        out=x_all[:, ck, :nblk],
                        in_=aps[f"x{s}"][ck * 128 : (ck + 1) * 128, n0 : n0 + nblk],
                    )
                _emit_ffn_block(
                    nc, pools, x_all, wg_sb, wu_sb, wd_sb, sc_sb, aps[f"y{s}"], n0, nblk
                )

    nc.compile()
    return nc


def _run(nc, in_maps, tag):
    from concourse.bass_utils import run_bass_kernel_spmd

    if TRACE:
        _enable_axon_ntff_profiling()
        res = run_bass_kernel_spmd(nc, in_maps, list(range(NCORE)), trace=True)
        LAST[f"{tag}_ns"] = res.exec_time_ns
        if res.instructions_and_trace is not None:
            LAST[f"{tag}_trace"] = res.instructions_and_trace[1]
    else:
        res = run_bass_kernel_spmd(nc, in_maps, list(range(NCORE)), trace=False)
    return res.results


def kernel(x, w_gate, expert_bias, wg, wu, wd, swg, swu, swd):
    LAST.clear()
    xf = np.ascontiguousarray(np.asarray(x, np.float32).reshape(NTOK, C))
    w_gate = np.ascontiguousarray(np.asarray(w_gate, np.float32))
    expert_bias = np.asarray(expert_bias, np.float32)
    wg = np.asarray(wg, np.float32)
    wu = np.asarray(wu, np.float32)
    wd = np.asarray(wd, np.float32)
    swg = np.ascontiguousarray(np.asarray(swg, np.float32))
    swu = np.ascontiguousarray(np.asarray(swu, np.float32))
    swd = np.ascontiguousarray(np.asarray(swd, np.float32))

    xt_full = np.ascontiguousarray(xf.T)  # (C, NTOK)
    bias_b = np.ascontiguousarray(np.broadcast_to(expert_bias, (128, E)))

    # ---- launch A: router + shared expert
    if "A" not in _progs:
        _progs["A"] = _build_launch_a()
    in_maps = []
    for c in range(NCORE):
        in_maps.append(
            {
                "xt": np.ascontiguousarray(xt_full[:, c * TPC : (c + 1) * TPC]),
                "wgate": w_gate,
                "biasb": bias_b,
                "swg": swg,
                "swu": swu,
                "swd": swd,
            }
        )
    res_a = _run(_progs["A"], in_maps, "launchA")

    comb = np.concatenate([res_a[c]["comb"] for c in range(NCORE)], axis=0)

    # ---- host routing: per-expert index lists + weights
    idxs, wts = [], []
    for e in range(E):
        ii = np.nonzero(comb[:, e] > 0.0)[0]
        idxs.append(ii)
        wts.append(comb[ii, e].astype(np.float32))
    max_cnt = max(len(ii) for ii in idxs)
    cap = max(NBLK, ((max_cnt + 127) // 128) * 128)

    # ---- launch B: routed experts (2 per core)
    key = ("B", cap)
    if key not in _progs:
        _progs[key] = _build_launch_b(cap)
    in_maps_b = []
    for c in range(NCORE):
        m = {}
        for s, e in (("a", 2 * c), ("b", 2 * c + 1)):
            ii, ww = idxs[e], wts[e]
            xt = np.zeros((C, cap), np.float32)
            xt[:, : len(ii)] = xf[ii].T
            sc = np.zeros((128, cap), np.float32)
            sc[:, : len(ii)] = ww[None, :]
            m[f"x{s}t"] = xt
            m[f"sc{s}"] = sc
            m[f"wg{s}"] = np.ascontiguousarray(wg[e])
            m[f"wu{s}"] = np.ascontiguousarray(wu[e])
            m[f"wd{s}"] = np.ascontiguousarray(wd[e])
        in_maps_b.append(m)
    res_b = _run(_progs[key], in_maps_b, "launchB")

    # ---- host combine: shared + scattered weighted expert outputs
    out = np.empty((NTOK, C), np.float32)
    for c in range(NCORE):
        out[c * TPC : (c + 1) * TPC] = res_a[c]["yst"].T
    for e in range(E):
        c, s = e // 2, ("a", "b")[e % 2]
        y = res_b[c][f"y{s}t"]  # (C, cap), already comb-weighted
        out[idxs[e]] += y[:, : len(idxs[e])].T

    if TRACE:
        LAST["total_ns"] = sum(
            v for k, v in LAST.items() if isinstance(v, int) and k.endswith("_ns")
        )
    return out.reshape(B, T, C)



# revision 2
# speedup vs baseline: 1.2752x; 1.2752x over previous
"""MoE (16 routed experts, top-2, + shared expert) on 8 TRN2 NeuronCores.

Strategy (expert-parallel, single fused launch):
  Host: computes the router (fp32 logits/softmax/top-2 — bit-matches the
    jax reference selection), gathers each expert's tokens into dense
    feature-major batches (the all-to-all dispatch), casts everything to
    bf16, and pairs experts onto cores largest-with-smallest so all 8
    cores carry the same padded token load.
  Device (one SPMD launch, all 8 cores): three SwiGLU FFN streams per
    core — the shared expert over its 2048-token data-parallel slice,
    then its two routed experts over their gathered batches.  bf16
    operands (fp32 PSUM accumulation) keep the PE at the 1 cyc/row peak
    while halving DMA vs fp32.  Weight sets are double-buffered so the
    next segment's weights stream in under the current segment's
    matmuls; the block loop is software-pipelined (gate/up of block i+1
    is emitted before down of block i) so the PE never waits on the
    vector engine.
  Host: combine — scales expert outputs by the renormalized top-2
    weights and scatter-adds them with the shared output into the full
    result.

All activations travel transposed (feature-major, token-minor) so every
matmul operand loads with natural DMA strides and zero on-device
transposes.
"""

import numpy as np

# model dims (fixed for this problem)
E, TOPK, C, I = 16, 2, 768, 1536
B, T = 8, 2048
NCORE = 8
NTOK = B * T           # 16384
TPC = NTOK // NCORE    # 2048 tokens per core (shared-expert DP slice)
CK = C // 128          # 6 contraction chunks for C
IK = I // 128          # 12 chunks for I
NBLK = 512             # token block = PE moving-dim per matmul

TRACE = False          # set True (from a driver) to capture NTFF timing
LAST = {}              # timing info from the most recent kernel() call

_progs = {}            # compiled program cache


def _enable_axon_ntff_profiling():
    import sys
    import types

    if "antenv.axon_hooks" not in sys.modules:
        mod = types.ModuleType("antenv.axon_hooks")
        mod._hook = None
        mod.set_axon_ntff_profile_hook = lambda h: setattr(mod, "_hook", h)
        mod.get_axon_ntff_profile_hook = lambda: mod._hook
        sys.modules["antenv.axon_hooks"] = mod
    from antenv.axon_hooks import set_axon_ntff_profile_hook  # type: ignore
    from trn_agent_boot.trn_boot import _ntff_profile_via_ctypes

    set_axon_ntff_profile_hook(_ntff_profile_via_ctypes("/opt/axon/libaxon_pjrt.so"))
    import concourse.bass_utils as bu

    bu.upload_artifacts = lambda tmpdir: f"file://{tmpdir}"


def _blocks(m):
    """Split m tokens into PE-friendly blocks (<=512 each)."""
    out = []
    n0 = 0
    while n0 < m:
        nb = min(NBLK, m - n0)
        out.append((n0, nb))
        n0 += nb
    return out


def _build_fused(cap_a, cap_b):
    """One launch: shared-expert FFN on the DP slice + two routed experts."""
    from contextlib import ExitStack

    import concourse.tile as tile
    from concourse import bacc, mybir

    f32 = mybir.dt.float32
    bf16 = mybir.dt.bfloat16

    nc = bacc.Bacc("TRN2", target_bir_lowering=False, debug=False)
    widths = {"s": TPC, "a": cap_a, "b": cap_b}
    aps = {}
    for s, w in widths.items():
        aps[f"x{s}"] = nc.dram_tensor(f"x{s}", [C, w], bf16, kind="ExternalInput").ap()
        aps[f"wg{s}"] = nc.dram_tensor(f"wg{s}", [C, I], bf16, kind="ExternalInput").ap()
        aps[f"wu{s}"] = nc.dram_tensor(f"wu{s}", [C, I], bf16, kind="ExternalInput").ap()
        aps[f"wd{s}"] = nc.dram_tensor(f"wd{s}", [I, C], bf16, kind="ExternalInput").ap()
        aps[f"y{s}"] = nc.dram_tensor(f"y{s}", [C, w], bf16, kind="ExternalOutput").ap()

    with tile.TileContext(nc) as tc, ExitStack() as ctx:
        wpool = ctx.enter_context(tc.tile_pool(name="weights", bufs=2))
        xpool = ctx.enter_context(tc.tile_pool(name="xp", bufs=3))
        hpool = ctx.enter_context(tc.tile_pool(name="hp", bufs=2))
        gpool = ctx.enter_context(tc.tile_pool(name="gp", bufs=2))
        ypool = ctx.enter_context(tc.tile_pool(name="yp", bufs=4))
        pgu = ctx.enter_context(tc.tile_pool(name="pgu", bufs=2, space="PSUM"))
        pd = ctx.enter_context(tc.tile_pool(name="pd", bufs=2, space="PSUM"))

        # flat task list across the three segments; weights double-buffer
        tasks = []
        for s, w in widths.items():
            wg_sb = wpool.tile([128, CK, I], bf16, tag="wg")
            wu_sb = wpool.tile([128, CK, I], bf16, tag="wu")
            wd_sb = wpool.tile([128, IK, C], bf16, tag="wd")
            for ck in range(CK):
                nc.sync.dma_start(
                    out=wg_sb[:, ck, :], in_=aps[f"wg{s}"][ck * 128 : (ck + 1) * 128, :]
                )
            for ck in range(CK):
                nc.sync.dma_start(
                    out=wu_sb[:, ck, :], in_=aps[f"wu{s}"][ck * 128 : (ck + 1) * 128, :]
                )
            for ik in range(IK):
                nc.sync.dma_start(
                    out=wd_sb[:, ik, :], in_=aps[f"wd{s}"][ik * 128 : (ik + 1) * 128, :]
                )
            for n0, nblk in _blocks(w):
                tasks.append((s, n0, nblk, wg_sb, wu_sb, wd_sb))

        def emit_gate_up(x_sb, wg_sb, wu_sb, h_sb, nblk):
            for ik in range(IK):
                psg = pgu.tile([128, NBLK], f32, tag="psg")
                psu = pgu.tile([128, NBLK], f32, tag="psu")
                for ck in range(CK):
                    nc.tensor.matmul(
                        psg[:, :nblk],
                        lhsT=wg_sb[:, ck, ik * 128 : (ik + 1) * 128],
                        rhs=x_sb[:, ck, :nblk],
                        start=(ck == 0),
                        stop=(ck == CK - 1),
                    )
                for ck in range(CK):
                    nc.tensor.matmul(
                        psu[:, :nblk],
                        lhsT=wu_sb[:, ck, ik * 128 : (ik + 1) * 128],
                        rhs=x_sb[:, ck, :nblk],
                        start=(ck == 0),
                        stop=(ck == CK - 1),
                    )
                ga = gpool.tile([128, NBLK], f32, tag="ga")
                nc.scalar.activation(
                    ga[:, :nblk], psg[:, :nblk], mybir.ActivationFunctionType.Silu
                )
                nc.vector.tensor_mul(h_sb[:, ik, :nblk], ga[:, :nblk], psu[:, :nblk])

        def emit_down(h_sb, wd_sb, y_ap, n0, nblk):
            for ck in range(CK):
                psd = pd.tile([128, NBLK], f32, tag="psd")
                for ik in range(IK):
                    nc.tensor.matmul(
                        psd[:, :nblk],
                        lhsT=wd_sb[:, ik, ck * 128 : (ck + 1) * 128],
                        rhs=h_sb[:, ik, :nblk],
                        start=(ik == 0),
                        stop=(ik == IK - 1),
                    )
                yb = ypool.tile([128, NBLK], bf16, tag="yb")
                nc.vector.tensor_copy(yb[:, :nblk], psd[:, :nblk])
                nc.sync.dma_start(
                    out=y_ap[ck * 128 : (ck + 1) * 128, n0 : n0 + nblk],
                    in_=yb[:, :nblk],
                )

        # software pipeline: gate/up of task i, then down of task i-1, so the
        # PE has matmul work while the vector engine finishes h of task i
        pending = None
        for s, n0, nblk, wg_sb, wu_sb, wd_sb in tasks:
            x_sb = xpool.tile([128, CK, NBLK], bf16, tag="x")
            for ck in range(CK):
                nc.sync.dma_start(
                    out=x_sb[:, ck, :nblk],
                    in_=aps[f"x{s}"][ck * 128 : (ck + 1) * 128, n0 : n0 + nblk],
                )
            h_sb = hpool.tile([128, IK, NBLK], bf16, tag="h")
            emit_gate_up(x_sb, wg_sb, wu_sb, h_sb, nblk)
            if pending is not None:
                emit_down(*pending)
            pending = (h_sb, wd_sb, aps[f"y{s}"], n0, nblk)
        emit_down(*pending)

    nc.compile()
    return nc


def _run(nc, in_maps, tag):
    from concourse.bass_utils import run_bass_kernel_spmd

    if TRACE:
        _enable_axon_ntff_profiling()
        res = run_bass_kernel_spmd(nc, in_maps, list(range(NCORE)), trace=True)
        LAST[f"{tag}_ns"] = res.exec_time_ns
        if res.instructions_and_trace is not None:
            LAST[f"{tag}_trace"] = res.instructions_and_trace[1]
    else:
        res = run_bass_kernel_spmd(nc, in_maps, list(range(NCORE)), trace=False)
    return res.results


def kernel(x, w_gate, expert_bias, wg, wu, wd, swg, swu, swd):
    import ml_dtypes

    bf16 = ml_dtypes.bfloat16
    LAST.clear()
    xf = np.asarray(x, np.float32).reshape(NTOK, C)
    w_gate = np.asarray(w_gate, np.float32)
    expert_bias = np.asarray(expert_bias, np.float32)

    # ---- host router (fp32, matches the reference's top-2 selection)
    logits = xf @ w_gate + expert_bias
    m = logits.max(-1, keepdims=True)
    ex = np.exp(logits - m, dtype=np.float32)
    probs = ex / ex.sum(-1, keepdims=True)
    ti = np.argpartition(-probs, TOPK - 1, axis=1)[:, :TOPK]  # unordered top-2
    tp = np.take_along_axis(probs, ti, axis=1)
    tp = tp / tp.sum(-1, keepdims=True)

    # per-expert token index lists + combine weights
    rows = np.repeat(np.arange(NTOK), TOPK)
    exps = ti.ravel()
    wts = tp.ravel().astype(np.float32)
    order = np.argsort(exps, kind="stable")
    rows, exps, wts = rows[order], exps[order], wts[order]
    starts = np.searchsorted(exps, np.arange(E + 1))
    idxs = [rows[starts[e] : starts[e + 1]] for e in range(E)]
    ews = [wts[starts[e] : starts[e + 1]] for e in range(E)]
    counts = np.array([len(ii) for ii in idxs])

    # pair experts onto cores: 8 largest in slot a, 8 smallest in slot b
    rank = np.argsort(-counts, kind="stable")
    slot_a = [int(rank[c]) for c in range(NCORE)]
    slot_b = [int(rank[2 * NCORE - 1 - c]) for c in range(NCORE)]
    cap_a = int(-(-counts[rank[0]] // 128) * 128)
    cap_b = int(-(-counts[rank[NCORE]] // 128) * 128)

    # ---- bf16 staging
    xT = np.ascontiguousarray(xf.T).astype(bf16)  # (C, NTOK) feature-major
    wg_bf = np.asarray(wg, np.float32).astype(bf16)
    wu_bf = np.asarray(wu, np.float32).astype(bf16)
    wd_bf = np.asarray(wd, np.float32).astype(bf16)
    swg_bf = np.asarray(swg, np.float32).astype(bf16)
    swu_bf = np.asarray(swu, np.float32).astype(bf16)
    swd_bf = np.asarray(swd, np.float32).astype(bf16)

    key = (cap_a, cap_b)
    if key not in _progs:
        _progs[key] = _build_fused(cap_a, cap_b)

    in_maps = []
    for c in range(NCORE):
        mcore = {
            "xs": np.ascontiguousarray(xT[:, c * TPC : (c + 1) * TPC]),
            "wgs": swg_bf,
            "wus": swu_bf,
            "wds": swd_bf,
        }
        for s, eidx, cap in (("a", slot_a[c], cap_a), ("b", slot_b[c], cap_b)):
            ii = idxs[eidx]
            xt = np.zeros((C, cap), bf16)
            xt[:, : len(ii)] = xT[:, ii]
            mcore[f"x{s}"] = xt
            mcore[f"wg{s}"] = wg_bf[eidx]
            mcore[f"wu{s}"] = wu_bf[eidx]
            mcore[f"wd{s}"] = wd_bf[eidx]
        in_maps.append(mcore)

    res = _run(_progs[key], in_maps, "fused")

    # ---- host combine: shared + weighted scattered expert outputs
    out = np.empty((NTOK, C), np.float32)
    for c in range(NCORE):
        out[c * TPC : (c + 1) * TPC] = res[c]["ys"].T.astype(np.float32)
    for c in range(NCORE):
        for s, eidx in (("a", slot_a[c]), ("b", slot_b[c])):
            ii = idxs[eidx]
            y = res[c][f"y{s}"][:, : len(ii)].T.astype(np.float32)
            out[ii] += ews[eidx][:, None] * y

    if TRACE:
        LAST["total_ns"] = sum(
            v for k, v in LAST.items() if isinstance(v, int) and k.endswith("_ns")
        )
    return out.reshape(B, T, C)
